# revision 22
# baseline (speedup 1.0000x reference)
"""Graphormer attention Trainium2 kernel (v2).

Problem: B=4, N=1024, D=256, H=8 heads (Dh=32), binned relative bias
  idx = clip(int(z/5*16), 0, 15);  scores = QK^T*scale + z_emb[idx]
  softmax over keys (key_mask additive -inf), out = attn @ V -> out_proj.

Sharding: 8 cores <- (batch b, query-row half). Each core computes rows
[half*512, half*512+512) of batch b for all 8 heads. No collectives;
host slices inputs / concatenates outputs.

Device algorithm (transposed layout, keys on partitions):
  S^T[k, q] accumulated in PSUM:
     QK part:  matmul(lhsT=K^T_h [32d,128k], rhs=Q^T_h [32d,512q]) (fp16)
   + bias part: 15 cumulative threshold masks M_t[k,q] = (idx >= t)
     (fp8, exact 0/1) accumulated via scaled-identity matmuls.
     Masks are PAIRED: 7 fp8 DoubleRow matmuls (2 thresholds each at
     0.5 cyc/row) + 1 plain fp8 matmul for t=15. The diagonal weight
     tiles are constants (z_emb baked) DMA'd from host, fp8-quantized
     with error feedback so the cumulative staircase stays exact to
     ~half an fp8 ulp.
     Bin indices idx are precomputed on host and shipped as exact fp16
     integers; masks are built on device by Vector+GpSimd is_ge.
  E^T = exp(S^T*scale + (z_emb[0,h] + keymask*-1e30))  ScalarE, fp16 out
  NUM^T[d|Z, q] += matmul(lhsT=V_aug[128k, 33], rhs=E^T); V col 32 = ones
     -> NUM row 32 = softmax denominator Z (deferred normalization).
  A^T = NUM^T * (1/Z broadcast via small selector matmul); 1/Z for all
     8 heads computed by ONE batched [8,512] reciprocal.
  out^T[dm, q] = Wo^T-matmul(A^T) + bo'  (bo' = Wo@bv + bo host-folded,
     valid because attention weights sum to 1)
  out = PE-transpose(out^T) -> DMA.
"""

import numpy as np

import concourse.bass as bass
import concourse.bacc as bacc
import concourse.mybir as mybir
import concourse.tile as tile
from concourse.bass_utils import run_bass_kernel_spmd
from concourse.masks import make_identity

B, N, D, H, DH = 4, 1024, 256, 8, 32
NB = 16
MAX_Z = 5.0
SCALE = DH ** (-0.5)
NCORES = 8
QR = N // 2  # query rows per core
P = 128
NPAIR = 7    # DoubleRow threshold pairs (t=1..14); t=15 is a single
F32 = mybir.dt.float32
F16 = mybir.dt.float16
F8 = mybir.dt.float8e4
F8NP = mybir.dt.np(F8)

_CACHE = {}


def _staircase_plan(z_emb: np.ndarray):
    """Plan the threshold staircase: which is_ge thresholds to keep (the
    z_emb[0] base is step t=0 with an all-ones mask), paired for fp8
    DoubleRow matmuls, with error-feedback fp8 quantized step heights.

    Returns (thresholds, q) with thresholds a list of kept t values
    (even length, t=0 first) and q [H, len(thresholds)] fp8-exact step
    heights in score (pre-scale) units.
    """
    z_emb = np.asarray(z_emb, dtype=np.float64)
    dval = np.diff(z_emb, axis=0)                     # [15, H] step heights
    mag = np.abs(dval).max(axis=1)                    # worst-case across heads
    # 16 steps (t=0 base + 15 deltas); pairing needs an even count, so
    # drop the 2 smallest deltas if the bias error they introduce is tiny
    order = np.argsort(mag)
    drop = set()
    if mag[order[0]] + mag[order[1]] <= 0.012:
        drop = {int(order[0]), int(order[1])}
    kept = [0] + [t + 1 for t in range(15) if t not in drop]
    assert len(kept) % 2 == 0
    # cumulative targets at each kept threshold, in /SCALE units
    cum = np.concatenate([z_emb[0:1, :], z_emb[0:1, :] + np.cumsum(dval, axis=0)],
                         axis=0) / SCALE              # [16, H] level values
    q = np.zeros((H, len(kept)), dtype=np.float64)
    for h in range(H):
        qcum = 0.0
        for i, t in enumerate(kept):
            want = np.float32(cum[t, h] - qcum)
            qv = float(np.asarray(want, dtype=np.float32).astype(F8NP))
            q[h, i] = qv
            qcum += qv
    return kept, q


def _build(z_emb: np.ndarray):
    """Build the (core-uniform) Bass program."""
    kept, _ = _staircase_plan(z_emb)
    NP = len(kept) // 2  # DoubleRow pairs per (head, key-chunk)
    nc = bacc.Bacc(trn_type="TRN2")

    xT = nc.dram_tensor("xT", [D, N], F16, kind="ExternalInput")
    xTq = nc.dram_tensor("xTq", [D, QR], F16, kind="ExternalInput")
    # host-precomputed threshold mask pairs (fp8 0/1)
    mkpd = nc.dram_tensor("mkpd", [NP * N, 2 * QR], F8, kind="ExternalInput")
    wqT = nc.dram_tensor("wqT", [D, D], F16, kind="ExternalInput")
    wkT = nc.dram_tensor("wkT", [D, D], F16, kind="ExternalInput")
    wvT = nc.dram_tensor("wvT", [D, D], F16, kind="ExternalInput")
    woT = nc.dram_tensor("woT", [D, D], F16, kind="ExternalInput")
    kmadd = nc.dram_tensor("kmadd", [N, 1], F32, kind="ExternalInput")
    selhd = nc.dram_tensor("selhd", [8, H * 32], F32, kind="ExternalInput")
    boT = nc.dram_tensor("boT", [D, 1], F32, kind="ExternalInput")
    dgp = nc.dram_tensor("dgp", [H * NP * P, 2 * P], F8, kind="ExternalInput")
    out = nc.dram_tensor("out", [QR, D], F32, kind="ExternalOutput")

    NKC = N // P   # 8 key chunks
    NDC = D // P   # 2 d_model chunks

    with tile.TileContext(nc) as tc:
        with (
            tc.tile_pool(name="const", bufs=1) as const,
            tc.tile_pool(name="win", bufs=1) as win,
            tc.tile_pool(name="acts", bufs=1) as acts,
            tc.tile_pool(name="masks", bufs=1) as maskp,
            tc.tile_pool(name="epool", bufs=6) as epool,
            tc.tile_pool(name="misc", bufs=1) as misc,
            tc.tile_pool(name="outp", bufs=1) as outp,
            # PSUM budget: psc 3 tags + pnum 4 tags + pmisc 1 = 8 banks
            tc.tile_pool(name="psc", bufs=1, space="PSUM") as psc,
            tc.tile_pool(name="pnum", bufs=1, space="PSUM") as pnum,
            tc.tile_pool(name="pmisc", bufs=1, space="PSUM") as pmisc,
        ):
            # ---------------- constants ----------------
            ident32 = const.tile([P, P], F32, tag="i32", name="i32")
            make_identity(nc, ident32[:])
            # head-row selector for 1/Z broadcast: sel[h, 32h:32h+32] = 1
            selh = const.tile([8, H * 32], F32, tag="selh", name="selh")
            nc.sync.dma_start(selh[:], selhd[:])

            # ---------------- input DMAs ----------------
            xT_sb, xTq_sb = [], []
            for c in range(NDC):
                t = win.tile([P, N], F16, tag=f"xt{c}", name=f"xt{c}")
                nc.sync.dma_start(t[:], xT[c * P:(c + 1) * P, :])
                xT_sb.append(t)
                t = win.tile([P, QR], F16, tag=f"xtq{c}", name=f"xtq{c}")
                nc.sync.dma_start(t[:], xTq[c * P:(c + 1) * P, :])
                xTq_sb.append(t)
            w_sb = {}
            for name, dram in (("q", wqT), ("k", wkT), ("v", wvT), ("o", woT)):
                for c in range(NDC):
                    t = win.tile([P, D], F16, tag=f"w{name}{c}", name=f"w{name}{c}")
                    nc.sync.dma_start(t[:], dram[c * P:(c + 1) * P, :])
                    w_sb[name, c] = t
            km_sb = []
            for kc in range(NKC):
                t = win.tile([P, 1], F32, tag=f"km{kc}", name=f"km{kc}")
                nc.sync.dma_start(t[:], kmadd[kc * P:(kc + 1) * P, :])
                km_sb.append(t)
            boT_sb = []
            for c in range(NDC):
                t = win.tile([P, 1], F32, tag=f"bo{c}", name=f"bo{c}")
                nc.sync.dma_start(t[:], boT[c * P:(c + 1) * P, :])
                boT_sb.append(t)
            # fp8 diagonal staircase weights (constants given z_emb)
            dgp_sb = {}
            for h in range(H):
                for j in range(NP):
                    t = win.tile([P, 2, P], F8, tag=f"dgp{h}_{j}", name=f"dgp{h}_{j}")
                    r0 = (h * NP + j) * P
                    nc.sync.dma_start(
                        t[:].rearrange("p two f -> p (two f)"),
                        dgp[r0:r0 + P, :],
                    )
                    dgp_sb[h, j] = t

            # ---------------- projections ----------------
            KT_sb = [acts.tile([DH, N], F16, tag=f"kth{h}", name=f"kth{h}") for h in range(H)]
            QT_sb = [acts.tile([DH, QR], F16, tag=f"qth{h}", name=f"qth{h}") for h in range(H)]
            for hc in range(NDC):
                for nb in range(N // 512):
                    ps = pmisc.tile([P, 512], F32, tag="pm", name="pm")
                    for dc in range(NDC):
                        nc.tensor.matmul(
                            ps[:],
                            w_sb["k", dc][:, hc * P:(hc + 1) * P],
                            xT_sb[dc][:, nb * 512:(nb + 1) * 512],
                            start=(dc == 0), stop=(dc == NDC - 1),
                        )
                    for hr in range(4):
                        nc.scalar.copy(
                            KT_sb[4 * hc + hr][:, nb * 512:(nb + 1) * 512],
                            ps[32 * hr:32 * hr + 32, :],
                        )
                ps = pmisc.tile([P, QR], F32, tag="pm", name="pm")
                for dc in range(NDC):
                    nc.tensor.matmul(
                        ps[:],
                        w_sb["q", dc][:, hc * P:(hc + 1) * P],
                        xTq_sb[dc][:],
                        start=(dc == 0), stop=(dc == NDC - 1),
                    )
                for hr in range(4):
                    nc.scalar.copy(
                        QT_sb[4 * hc + hr][:], ps[32 * hr:32 * hr + 32, :]
                    )

            # V_aug[k, 33h+d] fp16, col 33h+32 = ones
            V_sb = [acts.tile([P, 33 * H], F16, tag=f"v{kc}", name=f"v{kc}") for kc in range(NKC)]
            for kc in range(NKC):
                ps = pmisc.tile([P, D], F32, tag="pm", name="pm")
                for dc in range(NDC):
                    nc.tensor.matmul(
                        ps[:],
                        xT_sb[dc][:, kc * P:(kc + 1) * P],
                        w_sb["v", dc][:],
                        start=(dc == 0), stop=(dc == NDC - 1),
                    )
                v3 = V_sb[kc][:].rearrange("p (h x) -> p h x", x=33)
                nc.scalar.copy(
                    v3[:, :, 0:32], ps[:].rearrange("p (h d) -> p h d", d=DH)
                )
                nc.vector.memset(v3[:, :, 32:33], 1.0)

            # NUM psum: 4 banks, 2 heads per bank at row offsets 0/64
            num_ps = [pnum.tile([P, QR], F32, tag=f"num{j}", name=f"num{j}") for j in range(4)]

            def num_slice(h, rows):
                j, i = divmod(h, 2)
                return num_ps[j][64 * i: 64 * i + rows, :]

            # ---------------- mask DMAs (front-loaded) -------------------
            mkp = {}
            for kc in range(NKC):
                for j in range(NP):
                    m = maskp.tile([P, 2, QR], F8, tag=f"mkp{kc}_{j}", name=f"mkp{kc}_{j}")
                    r0 = j * N + kc * P
                    nc.sync.dma_start(
                        m[:].rearrange("p two f -> p (two f)"),
                        mkpd[r0:r0 + P, :],
                    )
                    mkp[kc, j] = m

            # ---------------- main loop: groups of key chunks ------------
            for kcs in ([0, 1, 2], [3, 4, 5], [6, 7]):
                # per head: scores + bias -> exp -> NUM accumulate
                for h in range(H):
                    sc = {}
                    for gi, kc in enumerate(kcs):
                        ps = psc.tile([P, QR], F32, tag=f"sc{gi}", name=f"sc{gi}")
                        nc.tensor.matmul(
                            ps[:],
                            KT_sb[h][:, kc * P:(kc + 1) * P],
                            QT_sb[h][:],
                            start=True, stop=False,
                        )
                        sc[kc] = ps
                    # kc-inner so the stationary fp8 diag is reused
                    for j in range(NP):
                        for kc in kcs:
                            nc.tensor.matmul(
                                sc[kc][:], dgp_sb[h, j][:], mkp[kc, j][:],
                                start=False, stop=(j == NP - 1),
                                perf_mode=mybir.MatmulPerfMode.DoubleRow,
                            )
                    for kc in kcs:
                        e = epool.tile([P, QR], F16, tag="e", name="e")
                        nc.scalar.activation(
                            e[:], sc[kc][:], mybir.ActivationFunctionType.Exp,
                            bias=km_sb[kc][:], scale=float(SCALE),
                        )
                        nc.tensor.matmul(
                            num_slice(h, 33),
                            V_sb[kc][:, 33 * h: 33 * h + 33],
                            e[:],
                            start=(kc == 0), stop=(kc == NKC - 1),
                        )

            # ---------------- normalize + out-projection ----------------
            # gather all 8 denominators -> one batched reciprocal.
            # Engines can't write partition base 1..7, so stage each row
            # at partition 0 and scatter with tiny SBUF->SBUF DMAs.
            zall = misc.tile([8, QR], F32, tag="zall", name="zall")
            for h in range(H):
                zr = misc.tile([1, QR], F32, tag=f"zr{h}", name=f"zr{h}")
                nc.scalar.copy(zr[:], num_slice(h, 33)[32:33, :])
                nc.sync.dma_start(zall[h:h + 1, :], zr[:])
            zeps = misc.tile([8, QR], F32, tag="zeps", name="zeps")
            nc.vector.tensor_scalar(
                zeps[:], zall[:], 1e-30, None, op0=mybir.AluOpType.add,
            )
            zinv = misc.tile([8, QR], F32, tag="zinv", name="zinv")
            nc.vector.reciprocal(zinv[:], zeps[:])

            An = [outp.tile([P, QR], F16, tag=f"an{c}", name=f"an{c}") for c in range(NDC)]
            for h in range(H):
                hc, hr = divmod(h, 4)
                rsl = slice(32 * hr, 32 * hr + 32)
                rp = pmisc.tile([32, QR], F32, tag="pm", name="pm")
                nc.tensor.matmul(
                    rp[:], selh[:, 32 * h:32 * h + 32], zinv[:],
                    start=True, stop=True,
                )
                rp_sb = misc.tile([32, QR], F32, tag="rp_sb", name="rp_sb")
                nc.scalar.copy(rp_sb[:], rp[:])
                nc.vector.tensor_tensor(
                    An[hc][rsl, :], num_slice(h, 32), rp_sb[:],
                    op=mybir.AluOpType.mult,
                )

            oT = []
            for mc in range(NDC):
                ps = pmisc.tile([P, QR], F32, tag="pm", name="pm")
                for cc in range(NDC):
                    nc.tensor.matmul(
                        ps[:],
                        w_sb["o", cc][:, mc * P:(mc + 1) * P],
                        An[cc][:],
                        start=(cc == 0), stop=(cc == NDC - 1),
                    )
                ot = outp.tile([P, QR], F32, tag=f"ot{mc}", name=f"ot{mc}")
                nc.scalar.add(ot[:], ps[:], boT_sb[mc][:])
                oT.append(ot)

            # transpose out^T [dm, q] -> out [q, dm] and DMA
            for qb in range(QR // P):
                osb = outp.tile([P, D], F32, tag="osb", name="osb")
                for mc in range(NDC):
                    tp = pmisc.tile([P, P], F32, tag="pm", name="pm")
                    nc.tensor.transpose(
                        tp[:], oT[mc][:, qb * P:(qb + 1) * P], ident32[:]
                    )
                    nc.scalar.copy(osb[:, mc * P:(mc + 1) * P], tp[:])
                nc.sync.dma_start(out[qb * P:(qb + 1) * P, :], osb[:])

    if not nc.is_finalized():
        nc.finalize()
    return nc


def _prep_inputs(x, z_matrix, key_mask, Wq, bq, Wk, bk, Wv, bv, Wo, bo, z_emb,
                 **_unused):
    f32, f16 = np.float32, np.float16
    assert np.all(np.asarray(bq) == 0) and np.all(np.asarray(bk) == 0), (
        "nonzero bq/bk not supported by this kernel build"
    )
    z_emb = np.asarray(z_emb, dtype=f32)
    wqT = np.ascontiguousarray(np.asarray(Wq).T.astype(f16))
    wkT = np.ascontiguousarray(np.asarray(Wk).T.astype(f16))
    wvT = np.ascontiguousarray(np.asarray(Wv).T.astype(f16))
    woT = np.ascontiguousarray(np.asarray(Wo).T.astype(f16))
    # attention weights sum to 1 -> bv folds into output bias exactly
    bo_eff = (np.asarray(Wo) @ np.asarray(bv) + np.asarray(bo)).astype(f32)
    boT = np.ascontiguousarray(bo_eff.reshape(D, 1))

    # fp8 staircase diagonals (error-feedback quantized)
    kept, q = _staircase_plan(z_emb)
    NP = len(kept) // 2
    dgp = np.zeros((H, NP, P, 2, P), dtype=np.float32)
    ii = np.arange(P)
    for h in range(H):
        for j in range(NP):
            dgp[h, j, ii, 0, ii] = q[h, 2 * j]
            dgp[h, j, ii, 1, ii] = q[h, 2 * j + 1]
    dgp = np.ascontiguousarray(dgp.reshape(H * NP * P, 2 * P)).astype(F8NP)
    selhd = np.zeros((8, H * 32), dtype=f32)
    for h in range(H):
        selhd[h, 32 * h:32 * h + 32] = 1.0

    in_maps = []
    for c in range(NCORES):
        b, half = divmod(c, 2)
        q0 = half * QR
        xb = np.asarray(x[b], dtype=f32)                    # [N, D]
        xT_ = np.ascontiguousarray(xb.T.astype(f16))        # [D, N]
        xTq_ = np.ascontiguousarray(xb[q0:q0 + QR, :].T.astype(f16))
        # threshold masks from bin indices, shipped as fp8 0/1
        zb_f = np.asarray(z_matrix[b], dtype=f32) * np.float32(NB / MAX_Z)
        zb_i = np.clip(zb_f.astype(np.int32), 0, NB - 1)
        idxT = zb_i.T[:, q0:q0 + QR]                        # [N, QR] int32
        one = np.uint8(np.float32(1.0).astype(F8NP).view(np.uint8))
        mkp_u8 = np.zeros((NP, N, 2, QR), dtype=np.uint8)
        for j in range(NP):
            mkp_u8[j, :, 0, :][idxT >= kept[2 * j]] = one
            mkp_u8[j, :, 1, :][idxT >= kept[2 * j + 1]] = one
        mkpd = np.ascontiguousarray(
            mkp_u8.reshape(NP * N, 2 * QR)
        ).view(F8NP)
        kma = np.ascontiguousarray(
            (np.asarray(key_mask[b]).astype(f32) * np.float32(-1e30)).reshape(N, 1)
        )
        in_maps.append({
            "xT": xT_, "xTq": xTq_, "mkpd": mkpd,
            "wqT": wqT, "wkT": wkT, "wvT": wvT, "woT": woT,
            "kmadd": kma, "boT": boT,
            "dgp": dgp, "selhd": selhd,
        })
    return in_maps


def kernel(**inputs) -> np.ndarray:
    z_emb = np.asarray(inputs["z_emb"], dtype=np.float32)
    key = z_emb.tobytes()
    if key not in _CACHE:
        _CACHE[key] = _build(z_emb)
    nc = _CACHE[key]

    in_maps = _prep_inputs(**inputs)
    res = run_bass_kernel_spmd(nc, in_maps, core_ids=list(range(NCORES)))
    full = np.empty((B, N, D), dtype=np.float32)
    for c in range(NCORES):
        b, half = divmod(c, 2)
        full[b, half * QR:(half + 1) * QR, :] = res.results[c]["out"]
    return full


# revision 36
# speedup vs baseline: 1.0831x; 1.0831x over previous
"""Graphormer attention Trainium2 kernel (v2).

Problem: B=4, N=1024, D=256, H=8 heads (Dh=32), binned relative bias
  idx = clip(int(z/5*16), 0, 15);  scores = QK^T*scale + z_emb[idx]
  softmax over keys (key_mask additive -inf), out = attn @ V -> out_proj.

Sharding: 8 cores <- (batch b, query-row half). Each core computes rows
[half*512, half*512+512) of batch b for all 8 heads. No collectives;
host slices inputs / concatenates outputs.

Device algorithm (transposed layout, keys on partitions):
  S^T[k, q] accumulated in PSUM:
     QK part:  matmul(lhsT=K^T_h [32d,128k], rhs=Q^T_h [32d,512q]) (fp16)
   + bias part: 15 cumulative threshold masks M_t[k,q] = (idx >= t)
     (fp8, exact 0/1) accumulated via scaled-identity matmuls.
     Masks are PAIRED: 7 fp8 DoubleRow matmuls (2 thresholds each at
     0.5 cyc/row) + 1 plain fp8 matmul for t=15. The diagonal weight
     tiles are constants (z_emb baked) DMA'd from host, fp8-quantized
     with error feedback so the cumulative staircase stays exact to
     ~half an fp8 ulp.
     Masks are precomputed on host from the bin indices and DMA'd in
     (engine elementwise ops with fp8 outputs hit a microcoded slow
     path, and DMA queues have plenty of headroom).
  E^T = exp(S^T*scale + (z_emb[0,h] + keymask*-1e30))  ScalarE, fp16 out
  NUM^T[d|Z, q] += matmul(lhsT=V_aug[128k, 33], rhs=E^T); V col 32 = ones
     -> NUM row 32 = softmax denominator Z (deferred normalization).
  A^T = NUM^T * (1/Z broadcast via small selector matmul); 1/Z for all
     8 heads computed by ONE batched [8,512] reciprocal.
  out^T[dm, q] = Wo^T-matmul(A^T) + bo'  (bo' = Wo@bv + bo host-folded,
     valid because attention weights sum to 1)
  out = PE-transpose(out^T) -> DMA.
"""

import numpy as np

import concourse.bass as bass
import concourse.bacc as bacc
import concourse.mybir as mybir
import concourse.tile as tile
from concourse.bass_utils import run_bass_kernel_spmd
from concourse.masks import make_identity

B, N, D, H, DH = 4, 1024, 256, 8, 32
NB = 16
MAX_Z = 5.0
SCALE = DH ** (-0.5)
NCORES = 8
QR = N // 2  # query rows per core
P = 128
NPAIR = 7    # DoubleRow threshold pairs (t=1..14); t=15 is a single
F32 = mybir.dt.float32
F16 = mybir.dt.float16
F8 = mybir.dt.float8e4
F8NP = mybir.dt.np(F8)

_CACHE = {}


def _staircase_plan(z_emb: np.ndarray):
    """Plan the threshold staircase (t=1..15 cumulative is_ge masks; the
    z_emb[0] base rides the exp's bias operand). Thresholds are paired
    (t1,t2)..(t13,t14) for fp8 DoubleRow matmuls; t15 is a plain fp8
    matmul. Step heights are error-feedback fp8 quantized so the
    cumulative staircase tracks the exact one to ~half an fp8 ulp.

    Returns (kept, q): kept = [1..15], q [H, 15] step heights in
    pre-scale score units.
    """
    z_emb = np.asarray(z_emb, dtype=np.float64)
    dval = np.diff(z_emb, axis=0) / SCALE             # [15, H]
    kept = list(range(1, 16))
    q = np.zeros((H, 15), dtype=np.float64)
    for h in range(H):
        exact_cum = 0.0
        qcum = 0.0
        for t in range(15):
            exact_cum += dval[t, h]
            want = np.float32(exact_cum - qcum)
            qv = float(np.asarray(want, dtype=np.float32).astype(F8NP))
            q[h, t] = qv
            qcum += qv
    return kept, q


def _build(z_emb: np.ndarray):
    """Build the (core-uniform) Bass program."""
    NP = NPAIR  # DoubleRow pairs per (head, key-chunk); + 1 single (t=15)
    nc = bacc.Bacc(trn_type="TRN2")

    xT = nc.dram_tensor("xT", [D, N], F16, kind="ExternalInput")
    xTq = nc.dram_tensor("xTq", [D, QR], F16, kind="ExternalInput")
    # host-precomputed threshold masks (fp8 0/1): pairs + the t=15 single
    mkpd = nc.dram_tensor("mkpd", [NP * N, 2 * QR], F8, kind="ExternalInput")
    mksd = nc.dram_tensor("mksd", [N, QR], F8, kind="ExternalInput")
    wqT = nc.dram_tensor("wqT", [D, D], F16, kind="ExternalInput")
    wkT = nc.dram_tensor("wkT", [D, D], F16, kind="ExternalInput")
    wvT = nc.dram_tensor("wvT", [D, D], F16, kind="ExternalInput")
    woT = nc.dram_tensor("woT", [D, D], F16, kind="ExternalInput")
    cball = nc.dram_tensor("cball", [H * N, 1], F32, kind="ExternalInput")
    selhd = nc.dram_tensor("selhd", [8, H * 32], F32, kind="ExternalInput")
    boT = nc.dram_tensor("boT", [D, 1], F32, kind="ExternalInput")
    dgp = nc.dram_tensor("dgp", [H * NP * P, 2 * P], F8, kind="ExternalInput")
    dgs = nc.dram_tensor("dgs", [H * P, P], F8, kind="ExternalInput")
    out = nc.dram_tensor("out", [QR, D], F32, kind="ExternalOutput")

    NKC = N // P   # 8 key chunks
    NDC = D // P   # 2 d_model chunks

    with tile.TileContext(nc) as tc:
        with (
            tc.tile_pool(name="const", bufs=1) as const,
            tc.tile_pool(name="win", bufs=1) as win,
            tc.tile_pool(name="acts", bufs=1) as acts,
            tc.tile_pool(name="masks", bufs=1) as maskp,
            tc.tile_pool(name="epool", bufs=6) as epool,
            tc.tile_pool(name="misc", bufs=1) as misc,
            tc.tile_pool(name="outp", bufs=1) as outp,
            # PSUM budget: psc 4 tags + pnum 4 tags = 8 banks
            tc.tile_pool(name="psc", bufs=1, space="PSUM") as psc,
            tc.tile_pool(name="pnum", bufs=1, space="PSUM") as pnum,
        ):
            # ------- input DMAs, ordered by when compute needs them ------
            # 1) projection inputs (first PE work)
            xT_sb, xTq_sb = [], []
            for c in range(NDC):
                t = win.tile([P, N], F16, tag=f"xt{c}", name=f"xt{c}")
                for nb in range(N // 512):
                    nc.sync.dma_start(
                        t[:, nb * 512:(nb + 1) * 512],
                        xT[c * P:(c + 1) * P, nb * 512:(nb + 1) * 512],
                    )
                xT_sb.append(t)
                t = win.tile([P, QR], F16, tag=f"xtq{c}", name=f"xtq{c}")
                nc.sync.dma_start(t[:], xTq[c * P:(c + 1) * P, :])
                xTq_sb.append(t)
            w_sb = {}
            for name, dram in (("k", wkT), ("q", wqT), ("v", wvT), ("o", woT)):
                for c in range(NDC):
                    t = win.tile([P, D], F16, tag=f"w{name}{c}", name=f"w{name}{c}")
                    nc.sync.dma_start(t[:], dram[c * P:(c + 1) * P, :])
                    w_sb[name, c] = t
            # 2) bias staircase weights + group-0 masks (first bias matmuls)
            dgp_sb, dgs_sb = {}, {}
            for h in range(H):
                for j in range(NP):
                    t = win.tile([P, 2, P], F8, tag=f"dgp{h}_{j}", name=f"dgp{h}_{j}")
                    r0 = (h * NP + j) * P
                    nc.sync.dma_start(
                        t[:].rearrange("p two f -> p (two f)"),
                        dgp[r0:r0 + P, :],
                    )
                    dgp_sb[h, j] = t
                t = win.tile([P, P], F8, tag=f"dgs{h}", name=f"dgs{h}")
                nc.sync.dma_start(t[:], dgs[h * P:(h + 1) * P, :])
                dgs_sb[h] = t

            mkp, mks = {}, {}

            def dma_masks(kc):
                for j in range(NP):
                    m = maskp.tile([P, 2, QR], F8, tag=f"mkp{kc}_{j}", name=f"mkp{kc}_{j}")
                    r0 = j * N + kc * P
                    nc.sync.dma_start(
                        m[:].rearrange("p two f -> p (two f)"),
                        mkpd[r0:r0 + P, :],
                    )
                    mkp[kc, j] = m
                m = maskp.tile([P, QR], F8, tag=f"mks{kc}", name=f"mks{kc}")
                nc.sync.dma_start(m[:], mksd[kc * P:(kc + 1) * P, :])
                mks[kc] = m

            for kc in (0, 1, 2, 3):
                dma_masks(kc)
            # 3) exp-bias rows (first exp comes after the first bias chain)
            cb = {}
            for h in range(H):
                for kc in range(NKC):
                    t = win.tile([P, 1], F32, tag=f"cb{h}_{kc}", name=f"cb{h}_{kc}")
                    nc.sync.dma_start(
                        t[:], cball[h * N + kc * P: h * N + (kc + 1) * P, :]
                    )
                    cb[h, kc] = t
            # 4) remaining masks and tail-phase constants
            for kc in range(4, NKC):
                dma_masks(kc)
            ident32 = const.tile([P, P], F32, tag="i32", name="i32")
            make_identity(nc, ident32[:])
            selh = const.tile([8, H * 32], F32, tag="selh", name="selh")
            nc.sync.dma_start(selh[:], selhd[:])
            boT_sb = []
            for c in range(NDC):
                t = win.tile([P, 1], F32, tag=f"bo{c}", name=f"bo{c}")
                nc.sync.dma_start(t[:], boT[c * P:(c + 1) * P, :])
                boT_sb.append(t)

            # ---------------- projections ----------------
            # scratch psum rotates over the 4 score banks (free until the
            # main loop) so head-split copies overlap the next matmul
            _scr = [0]

            def scratch_ps(cols):
                i = _scr[0] % 4
                _scr[0] += 1
                t = psc.tile([P, QR], F32, tag=f"sc{i}", name=f"sc{i}")
                return t[:, 0:cols]

            KT_sb = [acts.tile([DH, N], F16, tag=f"kth{h}", name=f"kth{h}") for h in range(H)]
            QT_sb = [acts.tile([DH, QR], F16, tag=f"qth{h}", name=f"qth{h}") for h in range(H)]
            for hc in range(NDC):
                for nb in range(N // 512):
                    ps = scratch_ps(512)
                    for dc in range(NDC):
                        nc.tensor.matmul(
                            ps[:],
                            w_sb["k", dc][:, hc * P:(hc + 1) * P],
                            xT_sb[dc][:, nb * 512:(nb + 1) * 512],
                            start=(dc == 0), stop=(dc == NDC - 1),
                        )
                    for hr in range(4):
                        nc.scalar.copy(
                            KT_sb[4 * hc + hr][:, nb * 512:(nb + 1) * 512],
                            ps[32 * hr:32 * hr + 32, :],
                        )
                ps = scratch_ps(QR)
                for dc in range(NDC):
                    nc.tensor.matmul(
                        ps[:],
                        w_sb["q", dc][:, hc * P:(hc + 1) * P],
                        xTq_sb[dc][:],
                        start=(dc == 0), stop=(dc == NDC - 1),
                    )
                for hr in range(4):
                    nc.scalar.copy(
                        QT_sb[4 * hc + hr][:], ps[32 * hr:32 * hr + 32, :]
                    )

            # V_aug[k, 33h+d] fp16, col 33h+32 = ones
            V_sb = [acts.tile([P, 33 * H], F16, tag=f"v{kc}", name=f"v{kc}") for kc in range(NKC)]
            for kc in range(NKC):
                ps = scratch_ps(D)
                for dc in range(NDC):
                    nc.tensor.matmul(
                        ps[:],
                        xT_sb[dc][:, kc * P:(kc + 1) * P],
                        w_sb["v", dc][:],
                        start=(dc == 0), stop=(dc == NDC - 1),
                    )
                v3 = V_sb[kc][:].rearrange("p (h x) -> p h x", x=33)
                nc.scalar.copy(
                    v3[:, :, 0:32], ps[:].rearrange("p (h d) -> p h d", d=DH)
                )
                nc.vector.memset(v3[:, :, 32:33], 1.0)

            # NUM psum: 4 banks, 2 heads per bank at row offsets 0/64
            num_ps = [pnum.tile([P, QR], F32, tag=f"num{j}", name=f"num{j}") for j in range(4)]

            def num_slice(h, rows):
                j, i = divmod(h, 2)
                return num_ps[j][64 * i: 64 * i + rows, :]

            # ---------------- main loop: groups of key chunks ------------
            # gather denominators into zall as each head's NUM finishes
            # (engines can't write partition base 1..7, so stage each row
            # at partition 0 and scatter with a tiny SBUF->SBUF DMA)
            zall = misc.tile([8, QR], F32, tag="zall", name="zall")
            for kcs in ([0, 1, 2, 3], [4, 5, 6, 7]):
                # per head: scores + bias -> exp -> NUM accumulate
                for h in range(H):
                    sc = {}
                    for gi, kc in enumerate(kcs):
                        ps = psc.tile([P, QR], F32, tag=f"sc{gi}", name=f"sc{gi}")
                        nc.tensor.matmul(
                            ps[:],
                            KT_sb[h][:, kc * P:(kc + 1) * P],
                            QT_sb[h][:],
                            start=True, stop=False,
                        )
                        sc[kc] = ps
                    # kc-inner so the stationary fp8 diag is reused
                    for j in range(NP):
                        for kc in kcs:
                            nc.tensor.matmul(
                                sc[kc][:], dgp_sb[h, j][:], mkp[kc, j][:],
                                start=False, stop=False,
                                perf_mode=mybir.MatmulPerfMode.DoubleRow,
                            )
                    for kc in kcs:
                        nc.tensor.matmul(
                            sc[kc][:], dgs_sb[h][:], mks[kc][:],
                            start=False, stop=True,
                        )
                    for kc in kcs:
                        e = epool.tile([P, QR], F16, tag="e", name="e")
                        nc.scalar.activation(
                            e[:], sc[kc][:], mybir.ActivationFunctionType.Exp,
                            bias=cb[h, kc][:], scale=float(SCALE),
                        )
                        nc.tensor.matmul(
                            num_slice(h, 33),
                            V_sb[kc][:, 33 * h: 33 * h + 33],
                            e[:],
                            start=(kc == 0), stop=(kc == NKC - 1),
                        )
                    if kcs[-1] == NKC - 1:
                        zr = misc.tile([1, QR], F32, tag=f"zr{h}", name=f"zr{h}")
                        nc.scalar.copy(zr[:], num_slice(h, 33)[32:33, :])
                        nc.sync.dma_start(zall[h:h + 1, :], zr[:])

            # ---------------- normalize + out-projection ----------------
            zeps = misc.tile([8, QR], F32, tag="zeps", name="zeps")
            nc.vector.tensor_scalar(
                zeps[:], zall[:], 1e-30, None, op0=mybir.AluOpType.add,
            )
            zinv = misc.tile([8, QR], F32, tag="zinv", name="zinv")
            nc.vector.reciprocal(zinv[:], zeps[:])

            An = [outp.tile([P, QR], F16, tag=f"an{c}", name=f"an{c}") for c in range(NDC)]
            for h in range(H):
                hc, hr = divmod(h, 4)
                rsl = slice(32 * hr, 32 * hr + 32)
                rp = scratch_ps(QR)[0:32, :]
                nc.tensor.matmul(
                    rp[:], selh[:, 32 * h:32 * h + 32], zinv[:],
                    start=True, stop=True,
                )
                rp_sb = misc.tile([32, QR], F32, tag="rp_sb", name="rp_sb")
                nc.scalar.copy(rp_sb[:], rp[:])
                nc.vector.tensor_tensor(
                    An[hc][rsl, :], num_slice(h, 32), rp_sb[:],
                    op=mybir.AluOpType.mult,
                )

            oT = []
            for mc in range(NDC):
                ps = scratch_ps(QR)
                for cc in range(NDC):
                    nc.tensor.matmul(
                        ps[:],
                        w_sb["o", cc][:, mc * P:(mc + 1) * P],
                        An[cc][:],
                        start=(cc == 0), stop=(cc == NDC - 1),
                    )
                ot = outp.tile([P, QR], F32, tag=f"ot{mc}", name=f"ot{mc}")
                nc.scalar.add(ot[:], ps[:], boT_sb[mc][:])
                oT.append(ot)

            # transpose out^T [dm, q] -> out [q, dm] and DMA
            for qb in range(QR // P):
                osb = outp.tile([P, D], F32, tag=f"osb{qb % 2}", name=f"osb{qb % 2}")
                for mc in range(NDC):
                    tp = scratch_ps(P)
                    nc.tensor.transpose(
                        tp[:], oT[mc][:, qb * P:(qb + 1) * P], ident32[:]
                    )
                    nc.scalar.copy(osb[:, mc * P:(mc + 1) * P], tp[:])
                nc.sync.dma_start(out[qb * P:(qb + 1) * P, :], osb[:])

    if not nc.is_finalized():
        nc.finalize()
    return nc


def _prep_inputs(x, z_matrix, key_mask, Wq, bq, Wk, bk, Wv, bv, Wo, bo, z_emb,
                 **_unused):
    f32, f16 = np.float32, np.float16
    assert np.all(np.asarray(bq) == 0) and np.all(np.asarray(bk) == 0), (
        "nonzero bq/bk not supported by this kernel build"
    )
    z_emb = np.asarray(z_emb, dtype=f32)
    wqT = np.ascontiguousarray(np.asarray(Wq).T.astype(f16))
    wkT = np.ascontiguousarray(np.asarray(Wk).T.astype(f16))
    wvT = np.ascontiguousarray(np.asarray(Wv).T.astype(f16))
    woT = np.ascontiguousarray(np.asarray(Wo).T.astype(f16))
    # attention weights sum to 1 -> bv folds into output bias exactly
    bo_eff = (np.asarray(Wo) @ np.asarray(bv) + np.asarray(bo)).astype(f32)
    boT = np.ascontiguousarray(bo_eff.reshape(D, 1))

    # fp8 staircase diagonals (error-feedback quantized)
    kept, q = _staircase_plan(z_emb)
    NP = NPAIR
    dgp = np.zeros((H, NP, P, 2, P), dtype=np.float32)
    dgs = np.zeros((H, P, P), dtype=np.float32)
    ii = np.arange(P)
    for h in range(H):
        for j in range(NP):
            dgp[h, j, ii, 0, ii] = q[h, 2 * j]
            dgp[h, j, ii, 1, ii] = q[h, 2 * j + 1]
        dgs[h, ii, ii] = q[h, 14]
    dgp = np.ascontiguousarray(dgp.reshape(H * NP * P, 2 * P)).astype(F8NP)
    dgs = np.ascontiguousarray(dgs.reshape(H * P, P)).astype(F8NP)
    selhd = np.zeros((8, H * 32), dtype=f32)
    for h in range(H):
        selhd[h, 32 * h:32 * h + 32] = 1.0

    in_maps = []
    for c in range(NCORES):
        b, half = divmod(c, 2)
        q0 = half * QR
        xb = np.asarray(x[b], dtype=f32)                    # [N, D]
        xT_ = np.ascontiguousarray(xb.T.astype(f16))        # [D, N]
        xTq_ = np.ascontiguousarray(xb[q0:q0 + QR, :].T.astype(f16))
        # threshold masks from bin indices, shipped as fp8 0/1
        zb_f = np.asarray(z_matrix[b], dtype=f32) * np.float32(NB / MAX_Z)
        zb_i = np.clip(zb_f.astype(np.int32), 0, NB - 1)
        idxT = zb_i.T[:, q0:q0 + QR]                        # [N, QR] int32
        one = np.uint8(np.float32(1.0).astype(F8NP).view(np.uint8))
        mkp_u8 = np.zeros((NP, N, 2, QR), dtype=np.uint8)
        for j in range(NP):
            mkp_u8[j, :, 0, :][idxT >= kept[2 * j]] = one
            mkp_u8[j, :, 1, :][idxT >= kept[2 * j + 1]] = one
        mkpd = np.ascontiguousarray(
            mkp_u8.reshape(NP * N, 2 * QR)
        ).view(F8NP)
        mks_u8 = np.zeros((N, QR), dtype=np.uint8)
        mks_u8[idxT >= kept[14]] = one
        mksd = np.ascontiguousarray(mks_u8).view(F8NP)
        # exp bias rows: keymask*-1e30 + z_emb[0, h]
        kma = np.asarray(key_mask[b]).astype(f32) * np.float32(-1e30)  # [N]
        cball = np.ascontiguousarray(
            (kma[None, :] + z_emb[0, :][:, None]).reshape(H * N, 1).astype(f32)
        )
        in_maps.append({
            "xT": xT_, "xTq": xTq_, "mkpd": mkpd, "mksd": mksd,
            "wqT": wqT, "wkT": wkT, "wvT": wvT, "woT": woT,
            "cball": cball, "boT": boT,
            "dgp": dgp, "dgs": dgs, "selhd": selhd,
        })
    return in_maps


def kernel(**inputs) -> np.ndarray:
    z_emb = np.asarray(inputs["z_emb"], dtype=np.float32)
    key = z_emb.tobytes()
    if key not in _CACHE:
        _CACHE[key] = _build(z_emb)
    nc = _CACHE[key]

    in_maps = _prep_inputs(**inputs)
    res = run_bass_kernel_spmd(nc, in_maps, core_ids=list(range(NCORES)))
    full = np.empty((B, N, D), dtype=np.float32)
    for c in range(NCORES):
        b, half = divmod(c, 2)
        full[b, half * QR:(half + 1) * QR, :] = res.results[c]["out"]
    return full


# revision 38
# speedup vs baseline: 1.2337x; 1.1390x over previous
"""Graphormer attention Trainium2 kernel (v2).

Problem: B=4, N=1024, D=256, H=8 heads (Dh=32), binned relative bias
  idx = clip(int(z/5*16), 0, 15);  scores = QK^T*scale + z_emb[idx]
  softmax over keys (key_mask additive -inf), out = attn @ V -> out_proj.

Sharding: 8 cores <- (batch b, query-row half). Each core computes rows
[half*512, half*512+512) of batch b for all 8 heads. No collectives;
host slices inputs / concatenates outputs.

Device algorithm (transposed layout, keys on partitions):
  S^T[k, q] accumulated in PSUM:
     QK part:  matmul(lhsT=K^T_h [32d,128k], rhs=Q^T_h [32d,512q]) (fp16)
   + bias part: 15 cumulative threshold masks M_t[k,q] = (idx >= t)
     (fp8, exact 0/1) accumulated via scaled-identity matmuls.
     Masks are PAIRED: 7 fp8 DoubleRow matmuls (2 thresholds each at
     0.5 cyc/row) + 1 plain fp8 matmul for t=15. The diagonal weight
     tiles are constants (z_emb baked) DMA'd from host, fp8-quantized
     with error feedback so the cumulative staircase stays exact to
     ~half an fp8 ulp.
     Masks are precomputed on host from the bin indices and DMA'd in
     (engine elementwise ops with fp8 outputs hit a microcoded slow
     path, and DMA queues have plenty of headroom).
  E^T = exp(S^T*scale + (z_emb[0,h] + keymask*-1e30))  ScalarE, fp16 out
  NUM^T[d|Z, q] += matmul(lhsT=V_aug[128k, 33], rhs=E^T); V col 32 = ones
     -> NUM row 32 = softmax denominator Z (deferred normalization).
  A^T = NUM^T * (1/Z broadcast via small selector matmul); 1/Z for all
     8 heads computed by ONE batched [8,512] reciprocal.
  out^T[dm, q] = Wo^T-matmul(A^T) + bo'  (bo' = Wo@bv + bo host-folded,
     valid because attention weights sum to 1)
  out = PE-transpose(out^T) -> DMA.
"""

import numpy as np

import concourse.bass as bass
import concourse.bacc as bacc
import concourse.mybir as mybir
import concourse.tile as tile
from concourse.bass_utils import run_bass_kernel_spmd
from concourse.masks import make_identity

B, N, D, H, DH = 4, 1024, 256, 8, 32
NB = 16
MAX_Z = 5.0
SCALE = DH ** (-0.5)
NCORES = 8
QR = N // 2  # query rows per core
P = 128
NPAIR = 7    # DoubleRow threshold pairs (t=1..14); t=15 is a single
F32 = mybir.dt.float32
F16 = mybir.dt.float16
F8 = mybir.dt.float8e4
F8NP = mybir.dt.np(F8)

_CACHE = {}


def _staircase_plan(z_emb: np.ndarray):
    """Plan the threshold staircase (t=1..15 cumulative is_ge masks; the
    z_emb[0] base rides the exp's bias operand). Thresholds are paired
    (t1,t2)..(t13,t14) for fp8 DoubleRow matmuls; t15 is a plain fp8
    matmul. Step heights are error-feedback fp8 quantized so the
    cumulative staircase tracks the exact one to ~half an fp8 ulp.

    Returns (kept, q): kept = [1..15], q [H, 15] step heights in
    pre-scale score units.
    """
    z_emb = np.asarray(z_emb, dtype=np.float64)
    dval = np.diff(z_emb, axis=0) / SCALE             # [15, H]
    kept = list(range(1, 16))
    q = np.zeros((H, 15), dtype=np.float64)
    for h in range(H):
        exact_cum = 0.0
        qcum = 0.0
        for t in range(15):
            exact_cum += dval[t, h]
            want = np.float32(exact_cum - qcum)
            qv = float(np.asarray(want, dtype=np.float32).astype(F8NP))
            q[h, t] = qv
            qcum += qv
    return kept, q


def _build(z_emb: np.ndarray):
    """Build the (core-uniform) Bass program."""
    NP = NPAIR  # DoubleRow pairs per (head, key-chunk); + 1 single (t=15)
    nc = bacc.Bacc(trn_type="TRN2")

    xT = nc.dram_tensor("xT", [D, N], F16, kind="ExternalInput")
    xTq = nc.dram_tensor("xTq", [D, QR], F16, kind="ExternalInput")
    # host-precomputed threshold masks (fp8 0/1): pairs + the t=15 single
    mkpd = nc.dram_tensor("mkpd", [NP * N, 2 * QR], F8, kind="ExternalInput")
    mksd = nc.dram_tensor("mksd", [N, QR], F8, kind="ExternalInput")
    wqT = nc.dram_tensor("wqT", [D, D], F16, kind="ExternalInput")
    wkT = nc.dram_tensor("wkT", [D, D], F16, kind="ExternalInput")
    wvT = nc.dram_tensor("wvT", [D, D], F16, kind="ExternalInput")
    woT = nc.dram_tensor("woT", [D, D], F16, kind="ExternalInput")
    cball = nc.dram_tensor("cball", [H * N, 1], F32, kind="ExternalInput")
    selhd = nc.dram_tensor("selhd", [4, 4 * 32], F32, kind="ExternalInput")
    boT = nc.dram_tensor("boT", [D, 1], F32, kind="ExternalInput")
    dgp = nc.dram_tensor("dgp", [H * NP * P, 2 * P], F8, kind="ExternalInput")
    dgs = nc.dram_tensor("dgs", [H * P, P], F8, kind="ExternalInput")
    out = nc.dram_tensor("out", [QR, D], F32, kind="ExternalOutput")

    NKC = N // P   # 8 key chunks
    NDC = D // P   # 2 d_model chunks

    with tile.TileContext(nc) as tc:
        with (
            tc.tile_pool(name="const", bufs=1) as const,
            tc.tile_pool(name="win", bufs=1) as win,
            tc.tile_pool(name="acts", bufs=1) as acts,
            tc.tile_pool(name="masks", bufs=1) as maskp,
            tc.tile_pool(name="epool", bufs=6) as epool,
            tc.tile_pool(name="misc", bufs=1) as misc,
            tc.tile_pool(name="outp", bufs=1) as outp,
            # PSUM budget: psc 4 tags + pnum 4 tags = 8 banks
            tc.tile_pool(name="psc", bufs=1, space="PSUM") as psc,
            tc.tile_pool(name="pnum", bufs=1, space="PSUM") as pnum,
        ):
            # ------- input DMAs, ordered by when compute needs them ------
            # 1) projection inputs (first PE work)
            xT_sb, xTq_sb = [], []
            for c in range(NDC):
                t = win.tile([P, N], F16, tag=f"xt{c}", name=f"xt{c}")
                for nb in range(N // 512):
                    nc.sync.dma_start(
                        t[:, nb * 512:(nb + 1) * 512],
                        xT[c * P:(c + 1) * P, nb * 512:(nb + 1) * 512],
                    )
                xT_sb.append(t)
                t = win.tile([P, QR], F16, tag=f"xtq{c}", name=f"xtq{c}")
                nc.sync.dma_start(t[:], xTq[c * P:(c + 1) * P, :])
                xTq_sb.append(t)
            w_sb = {}
            for name, dram in (("k", wkT), ("q", wqT), ("v", wvT), ("o", woT)):
                for c in range(NDC):
                    t = win.tile([P, D], F16, tag=f"w{name}{c}", name=f"w{name}{c}")
                    nc.sync.dma_start(t[:], dram[c * P:(c + 1) * P, :])
                    w_sb[name, c] = t
            # 2) bias staircase weights + group-0 masks (first bias matmuls)
            dgp_sb, dgs_sb = {}, {}
            for h in range(H):
                for j in range(NP):
                    t = win.tile([P, 2, P], F8, tag=f"dgp{h}_{j}", name=f"dgp{h}_{j}")
                    r0 = (h * NP + j) * P
                    nc.sync.dma_start(
                        t[:].rearrange("p two f -> p (two f)"),
                        dgp[r0:r0 + P, :],
                    )
                    dgp_sb[h, j] = t
                t = win.tile([P, P], F8, tag=f"dgs{h}", name=f"dgs{h}")
                nc.sync.dma_start(t[:], dgs[h * P:(h + 1) * P, :])
                dgs_sb[h] = t

            mkp, mks = {}, {}

            def dma_masks(kc):
                for j in range(NP):
                    m = maskp.tile([P, 2, QR], F8, tag=f"mkp{kc}_{j}", name=f"mkp{kc}_{j}")
                    r0 = j * N + kc * P
                    nc.sync.dma_start(
                        m[:].rearrange("p two f -> p (two f)"),
                        mkpd[r0:r0 + P, :],
                    )
                    mkp[kc, j] = m
                m = maskp.tile([P, QR], F8, tag=f"mks{kc}", name=f"mks{kc}")
                nc.sync.dma_start(m[:], mksd[kc * P:(kc + 1) * P, :])
                mks[kc] = m

            for kc in (0, 1, 2, 3):
                dma_masks(kc)
            # 3) exp-bias rows (first exp comes after the first bias chain)
            cb = {}
            for h in range(H):
                for kc in range(NKC):
                    t = win.tile([P, 1], F32, tag=f"cb{h}_{kc}", name=f"cb{h}_{kc}")
                    nc.sync.dma_start(
                        t[:], cball[h * N + kc * P: h * N + (kc + 1) * P, :]
                    )
                    cb[h, kc] = t
            # 4) remaining masks and tail-phase constants
            for kc in range(4, NKC):
                dma_masks(kc)
            ident32 = const.tile([P, P], F32, tag="i32", name="i32")
            make_identity(nc, ident32[:])
            selh = const.tile([4, 4 * 32], F32, tag="selh", name="selh")
            nc.sync.dma_start(selh[:], selhd[:])
            boT_sb = []
            for c in range(NDC):
                t = win.tile([P, 1], F32, tag=f"bo{c}", name=f"bo{c}")
                nc.sync.dma_start(t[:], boT[c * P:(c + 1) * P, :])
                boT_sb.append(t)

            # ---------------- projections ----------------
            # scratch psum rotates over the 4 score banks (free until the
            # main loop) so head-split copies overlap the next matmul
            _scr = [0]

            def scratch_ps(cols):
                i = _scr[0] % 4
                _scr[0] += 1
                t = psc.tile([P, QR], F32, tag=f"sc{i}", name=f"sc{i}")
                return t[:, 0:cols]

            def hcopy(dst, src_ap, i):
                # alternate head-split copies between Vector and Scalar
                if i % 2 == 0:
                    nc.vector.tensor_copy(dst, src_ap)
                else:
                    nc.scalar.copy(dst, src_ap)

            KT_sb = [acts.tile([DH, N], F16, tag=f"kth{h}", name=f"kth{h}") for h in range(H)]
            QT_sb = [acts.tile([DH, QR], F16, tag=f"qth{h}", name=f"qth{h}") for h in range(H)]
            for hc in range(NDC):
                for nb in range(N // 512):
                    ps = scratch_ps(512)
                    for dc in range(NDC):
                        nc.tensor.matmul(
                            ps[:],
                            w_sb["k", dc][:, hc * P:(hc + 1) * P],
                            xT_sb[dc][:, nb * 512:(nb + 1) * 512],
                            start=(dc == 0), stop=(dc == NDC - 1),
                        )
                    for hr in range(4):
                        hcopy(KT_sb[4 * hc + hr][:, nb * 512:(nb + 1) * 512],
                              ps[32 * hr:32 * hr + 32, :], hr)
                ps = scratch_ps(QR)
                for dc in range(NDC):
                    nc.tensor.matmul(
                        ps[:],
                        w_sb["q", dc][:, hc * P:(hc + 1) * P],
                        xTq_sb[dc][:],
                        start=(dc == 0), stop=(dc == NDC - 1),
                    )
                for hr in range(4):
                    hcopy(QT_sb[4 * hc + hr][:], ps[32 * hr:32 * hr + 32, :], hr)

            # V_aug[k, 33h+d] fp16, col 33h+32 = ones
            V_sb = [acts.tile([P, 33 * H], F16, tag=f"v{kc}", name=f"v{kc}") for kc in range(NKC)]
            for kc in range(NKC):
                ps = scratch_ps(D)
                for dc in range(NDC):
                    nc.tensor.matmul(
                        ps[:],
                        xT_sb[dc][:, kc * P:(kc + 1) * P],
                        w_sb["v", dc][:],
                        start=(dc == 0), stop=(dc == NDC - 1),
                    )
                v3 = V_sb[kc][:].rearrange("p (h x) -> p h x", x=33)
                nc.scalar.copy(
                    v3[:, :, 0:32], ps[:].rearrange("p (h d) -> p h d", d=DH)
                )
                nc.vector.memset(v3[:, :, 32:33], 1.0)

            # NUM psum: 4 banks, 2 heads per bank at row offsets 0/64
            num_ps = [pnum.tile([P, QR], F32, tag=f"num{j}", name=f"num{j}") for j in range(4)]

            def num_slice(h, rows):
                j, i = divmod(h, 2)
                return num_ps[j][64 * i: 64 * i + rows, :]

            # ---------------- main loop: groups of key chunks ------------
            # gather denominators into zall as each head's NUM finishes
            # (engines can't write partition base 1..7, so stage each row
            # at partition 0 and scatter with a tiny SBUF->SBUF DMA)
            zall_a = misc.tile([4, QR], F32, tag="zall_a", name="zall_a")
            zall_b = misc.tile([4, QR], F32, tag="zall_b", name="zall_b")
            zinv = {}
            for g, kcs in enumerate(([0, 1], [2, 3], [4, 5], [6, 7])):
                # per head: scores + bias -> exp -> NUM accumulate
                for h in range(H):
                    sc = {}
                    for gi, kc in enumerate(kcs):
                        tg = 2 * (g % 2) + gi
                        ps = psc.tile([P, QR], F32, tag=f"sc{tg}", name=f"sc{tg}")
                        nc.tensor.matmul(
                            ps[:],
                            KT_sb[h][:, kc * P:(kc + 1) * P],
                            QT_sb[h][:],
                            start=True, stop=False,
                        )
                        sc[kc] = ps
                    # kc-inner so the stationary fp8 diag is reused
                    for j in range(NP):
                        for kc in kcs:
                            nc.tensor.matmul(
                                sc[kc][:], dgp_sb[h, j][:], mkp[kc, j][:],
                                start=False, stop=False,
                                perf_mode=mybir.MatmulPerfMode.DoubleRow,
                            )
                    for kc in kcs:
                        nc.tensor.matmul(
                            sc[kc][:], dgs_sb[h][:], mks[kc][:],
                            start=False, stop=True,
                        )
                    for kc in kcs:
                        e = epool.tile([P, QR], F16, tag="e", name="e")
                        nc.scalar.activation(
                            e[:], sc[kc][:], mybir.ActivationFunctionType.Exp,
                            bias=cb[h, kc][:], scale=float(SCALE),
                        )
                        nc.tensor.matmul(
                            num_slice(h, 33),
                            V_sb[kc][:, 33 * h: 33 * h + 33],
                            e[:],
                            start=(kc == 0), stop=(kc == NKC - 1),
                        )
                    if kcs[-1] == NKC - 1:
                        zr = misc.tile([1, QR], F32, tag=f"zr{h}", name=f"zr{h}")
                        nc.scalar.copy(zr[:], num_slice(h, 33)[32:33, :])
                        ztile = zall_a if h < 4 else zall_b
                        nc.sync.dma_start(ztile[h % 4:h % 4 + 1, :], zr[:])
                        if h % 4 == 3:
                            # this half's denominators are complete:
                            # reciprocal overlaps the remaining heads
                            half = h // 4
                            zeps = misc.tile([4, QR], F32, tag=f"zeps{half}", name=f"zeps{half}")
                            nc.vector.tensor_scalar(
                                zeps[:], ztile[:], 1e-30, None,
                                op0=mybir.AluOpType.add,
                            )
                            zi = misc.tile([4, QR], F32, tag=f"zinv{half}", name=f"zinv{half}")
                            nc.vector.reciprocal(zi[:], zeps[:])
                            zinv[half] = zi

            # ---------------- normalize + out-projection ----------------
            An = [outp.tile([P, QR], F16, tag=f"an{c}", name=f"an{c}") for c in range(NDC)]
            for h in range(H):
                hc, hr = divmod(h, 4)
                rsl = slice(32 * hr, 32 * hr + 32)
                rp = scratch_ps(QR)[0:32, :]
                nc.tensor.matmul(
                    rp[:], selh[:, 32 * (h % 4):32 * (h % 4) + 32], zinv[h // 4][:],
                    start=True, stop=True,
                )
                rp_sb = misc.tile([32, QR], F32, tag="rp_sb", name="rp_sb")
                nc.vector.tensor_copy(rp_sb[:], rp[:])
                nc.vector.tensor_tensor(
                    An[hc][rsl, :], num_slice(h, 32), rp_sb[:],
                    op=mybir.AluOpType.mult,
                )

            oT = []
            for mc in range(NDC):
                ps = scratch_ps(QR)
                for cc in range(NDC):
                    nc.tensor.matmul(
                        ps[:],
                        w_sb["o", cc][:, mc * P:(mc + 1) * P],
                        An[cc][:],
                        start=(cc == 0), stop=(cc == NDC - 1),
                    )
                ot = outp.tile([P, QR], F32, tag=f"ot{mc}", name=f"ot{mc}")
                nc.scalar.add(ot[:], ps[:], boT_sb[mc][:])
                oT.append(ot)

            # transpose out^T [dm, q] -> out [q, dm] and DMA
            for qb in range(QR // P):
                osb = outp.tile([P, D], F32, tag=f"osb{qb % 2}", name=f"osb{qb % 2}")
                for mc in range(NDC):
                    tp = scratch_ps(P)
                    nc.tensor.transpose(
                        tp[:], oT[mc][:, qb * P:(qb + 1) * P], ident32[:]
                    )
                    hcopy(osb[:, mc * P:(mc + 1) * P], tp[:], mc)
                nc.sync.dma_start(out[qb * P:(qb + 1) * P, :], osb[:])

    if not nc.is_finalized():
        nc.finalize()
    return nc


def _prep_inputs(x, z_matrix, key_mask, Wq, bq, Wk, bk, Wv, bv, Wo, bo, z_emb,
                 **_unused):
    f32, f16 = np.float32, np.float16
    assert np.all(np.asarray(bq) == 0) and np.all(np.asarray(bk) == 0), (
        "nonzero bq/bk not supported by this kernel build"
    )
    z_emb = np.asarray(z_emb, dtype=f32)
    wqT = np.ascontiguousarray(np.asarray(Wq).T.astype(f16))
    wkT = np.ascontiguousarray(np.asarray(Wk).T.astype(f16))
    wvT = np.ascontiguousarray(np.asarray(Wv).T.astype(f16))
    woT = np.ascontiguousarray(np.asarray(Wo).T.astype(f16))
    # attention weights sum to 1 -> bv folds into output bias exactly
    bo_eff = (np.asarray(Wo) @ np.asarray(bv) + np.asarray(bo)).astype(f32)
    boT = np.ascontiguousarray(bo_eff.reshape(D, 1))

    # fp8 staircase diagonals (error-feedback quantized)
    kept, q = _staircase_plan(z_emb)
    NP = NPAIR
    dgp = np.zeros((H, NP, P, 2, P), dtype=np.float32)
    dgs = np.zeros((H, P, P), dtype=np.float32)
    ii = np.arange(P)
    for h in range(H):
        for j in range(NP):
            dgp[h, j, ii, 0, ii] = q[h, 2 * j]
            dgp[h, j, ii, 1, ii] = q[h, 2 * j + 1]
        dgs[h, ii, ii] = q[h, 14]
    dgp = np.ascontiguousarray(dgp.reshape(H * NP * P, 2 * P)).astype(F8NP)
    dgs = np.ascontiguousarray(dgs.reshape(H * P, P)).astype(F8NP)
    selhd = np.zeros((4, 4 * 32), dtype=f32)
    for h in range(4):
        selhd[h, 32 * h:32 * h + 32] = 1.0

    in_maps = []
    for c in range(NCORES):
        b, half = divmod(c, 2)
        q0 = half * QR
        xb = np.asarray(x[b], dtype=f32)                    # [N, D]
        xT_ = np.ascontiguousarray(xb.T.astype(f16))        # [D, N]
        xTq_ = np.ascontiguousarray(xb[q0:q0 + QR, :].T.astype(f16))
        # threshold masks from bin indices, shipped as fp8 0/1
        zb_f = np.asarray(z_matrix[b], dtype=f32) * np.float32(NB / MAX_Z)
        zb_i = np.clip(zb_f.astype(np.int32), 0, NB - 1)
        idxT = zb_i.T[:, q0:q0 + QR]                        # [N, QR] int32
        one = np.uint8(np.float32(1.0).astype(F8NP).view(np.uint8))
        mkp_u8 = np.zeros((NP, N, 2, QR), dtype=np.uint8)
        for j in range(NP):
            mkp_u8[j, :, 0, :][idxT >= kept[2 * j]] = one
            mkp_u8[j, :, 1, :][idxT >= kept[2 * j + 1]] = one
        mkpd = np.ascontiguousarray(
            mkp_u8.reshape(NP * N, 2 * QR)
        ).view(F8NP)
        mks_u8 = np.zeros((N, QR), dtype=np.uint8)
        mks_u8[idxT >= kept[14]] = one
        mksd = np.ascontiguousarray(mks_u8).view(F8NP)
        # exp bias rows: keymask*-1e30 + z_emb[0, h]
        kma = np.asarray(key_mask[b]).astype(f32) * np.float32(-1e30)  # [N]
        cball = np.ascontiguousarray(
            (kma[None, :] + z_emb[0, :][:, None]).reshape(H * N, 1).astype(f32)
        )
        in_maps.append({
            "xT": xT_, "xTq": xTq_, "mkpd": mkpd, "mksd": mksd,
            "wqT": wqT, "wkT": wkT, "wvT": wvT, "woT": woT,
            "cball": cball, "boT": boT,
            "dgp": dgp, "dgs": dgs, "selhd": selhd,
        })
    return in_maps


def kernel(**inputs) -> np.ndarray:
    z_emb = np.asarray(inputs["z_emb"], dtype=np.float32)
    key = z_emb.tobytes()
    if key not in _CACHE:
        _CACHE[key] = _build(z_emb)
    nc = _CACHE[key]

    in_maps = _prep_inputs(**inputs)
    res = run_bass_kernel_spmd(nc, in_maps, core_ids=list(range(NCORES)))
    full = np.empty((B, N, D), dtype=np.float32)
    for c in range(NCORES):
        b, half = divmod(c, 2)
        full[b, half * QR:(half + 1) * QR, :] = res.results[c]["out"]
    return full


# revision 39
# speedup vs baseline: 1.4258x; 1.1557x over previous
"""Graphormer attention Trainium2 kernel.

Problem: B=4, N=1024, D=256, H=8 heads (Dh=32), binned relative bias
  idx = clip(int(z/5*16), 0, 15);  scores = QK^T*scale + z_emb[idx]
  softmax over keys (key_mask additive -inf), out = attn @ V -> out_proj.

Sharding: 8 cores <- (batch b, query-row half). Each core computes rows
[half*512, half*512+512) of batch b for all 8 heads. No collectives;
host slices inputs / concatenates outputs.

Device algorithm (transposed layout, keys on partitions):
  S^T[k, q] accumulated in PSUM:
     QK part:  matmul(lhsT=K^T_h [32d,128k], rhs=Q^T_h [32d,512q]) (fp16)
   + bias part: 15 cumulative threshold masks M_t[k,q] = (idx >= t)
     (fp8, exact 0/1) accumulated via scaled-identity matmuls:
     7 fp8 DoubleRow matmuls (2 thresholds each; the fp8 double pump is
     spent on the extra contraction slot) + 1 plain fp8 matmul for t=15.
     Masks are precomputed on host from the bin indices; the diagonal
     step weights are error-feedback fp8 quantized so the cumulative
     staircase tracks the exact one to ~half an fp8 ulp.
  E^T = exp(S^T*scale + (z_emb[0,h] + keymask*-1e30))  ScalarE, fp16 out
  NUM^T[d|Z, q] += matmul(lhsT=V_aug[128k, 33], rhs=E^T); V col 32 = ones
     -> NUM row 32 = softmax denominator Z (deferred normalization).
  A^T = NUM^T * (1/Z broadcast via small selector matmul); 1/Z computed
     by two batched [4,512] reciprocals, each as soon as its half of the
     heads finishes.
  out^T[dm, q] = Wo^T-matmul(A^T) + bo'  (bo' = Wo@bv + bo host-folded,
     valid because attention weights sum to 1)
  out = PE-transpose(out^T) -> DMA.

DMA discipline: the DMA engines are descriptor-bound (~22ns per
partition-row descriptor), so every constant is shipped as ONE wide
per-partition-contiguous transfer: all 15 masks of a key chunk in one
[128, 15*512B] DMA, all 120 diagonal tiles in one [128, 15KB] DMA, all
8 weight tiles in one, all exp-bias rows + output bias in one.
"""

import numpy as np

import concourse.bass as bass
import concourse.bacc as bacc
import concourse.mybir as mybir
import concourse.tile as tile
from concourse.bass_utils import run_bass_kernel_spmd
from concourse.masks import make_identity

B, N, D, H, DH = 4, 1024, 256, 8, 32
NB = 16
MAX_Z = 5.0
SCALE = DH ** (-0.5)
NCORES = 8
QR = N // 2  # query rows per core
P = 128
NP = 7       # DoubleRow threshold pairs (t=1..14); t=15 is a single
NM = 15      # threshold masks
F32 = mybir.dt.float32
F16 = mybir.dt.float16
F8 = mybir.dt.float8e4
F8NP = mybir.dt.np(F8)

_CACHE = {}


def _staircase_q(z_emb: np.ndarray) -> np.ndarray:
    """fp8 step heights q[h, t] (t=1..15), error-feedback quantized so
    the cumulative staircase tracks the exact one, in pre-scale units."""
    dval = np.diff(np.asarray(z_emb, dtype=np.float64), axis=0) / SCALE
    q = np.zeros((H, NM), dtype=np.float64)
    for h in range(H):
        exact_cum = 0.0
        qcum = 0.0
        for t in range(NM):
            exact_cum += dval[t, h]
            want = np.float32(exact_cum - qcum)
            qv = float(np.asarray(want, dtype=np.float32).astype(F8NP))
            q[h, t] = qv
            qcum += qv
    return q


def _build(z_emb: np.ndarray):
    """Build the (core-uniform) Bass program."""
    nc = bacc.Bacc(trn_type="TRN2")

    xT = nc.dram_tensor("xT", [D, N], F16, kind="ExternalInput")
    xTq = nc.dram_tensor("xTq", [D, QR], F16, kind="ExternalInput")
    # all 15 masks of a key chunk concatenated per partition row
    mcatd = nc.dram_tensor("mcatd", [N, NM * QR], F8, kind="ExternalInput")
    # all (head, threshold) diagonal tiles concatenated per partition row
    dgalld = nc.dram_tensor("dgalld", [P, H * NM * P], F8, kind="ExternalInput")
    # q/k/v/o weight tiles concatenated per partition row
    wcatd = nc.dram_tensor("wcatd", [P, 8 * D], F16, kind="ExternalInput")
    # exp-bias rows (keymask*-1e30 + z_emb[0,h]) + folded output bias
    cbtd = nc.dram_tensor("cbtd", [P, H * 8 + 2], F32, kind="ExternalInput")
    selhd = nc.dram_tensor("selhd", [4, 4 * 32], F32, kind="ExternalInput")
    out = nc.dram_tensor("out", [QR, D], F32, kind="ExternalOutput")

    NKC = N // P   # 8 key chunks
    NDC = D // P   # 2 d_model chunks

    with tile.TileContext(nc) as tc:
        with (
            tc.tile_pool(name="const", bufs=1) as const,
            tc.tile_pool(name="win", bufs=1) as win,
            tc.tile_pool(name="acts", bufs=1) as acts,
            tc.tile_pool(name="masks", bufs=1) as maskp,
            tc.tile_pool(name="epool", bufs=6) as epool,
            tc.tile_pool(name="misc", bufs=1) as misc,
            tc.tile_pool(name="outp", bufs=1) as outp,
            # PSUM budget: psc 4 tags + pnum 4 tags = 8 banks
            tc.tile_pool(name="psc", bufs=1, space="PSUM") as psc,
            tc.tile_pool(name="pnum", bufs=1, space="PSUM") as pnum,
        ):
            # ------- input DMAs, ordered by when compute needs them ------
            xT_sb, xTq_sb = [], []
            for c in range(NDC):
                t = win.tile([P, N], F16, tag=f"xt{c}", name=f"xt{c}")
                nc.sync.dma_start(t[:], xT[c * P:(c + 1) * P, :])
                xT_sb.append(t)
                t = win.tile([P, QR], F16, tag=f"xtq{c}", name=f"xtq{c}")
                nc.sync.dma_start(t[:], xTq[c * P:(c + 1) * P, :])
                xTq_sb.append(t)
            wall = win.tile([P, 8, D], F16, tag="wall", name="wall")
            nc.sync.dma_start(wall[:].rearrange("p i m -> p (i m)"), wcatd[:])
            w_sb = {}
            for i, name in enumerate(("k", "q", "v", "o")):
                for c in range(NDC):
                    w_sb[name, c] = wall[:, 2 * i + c, :]
            dgall = win.tile([P, H * NM, P], F8, tag="dgall", name="dgall")
            nc.sync.dma_start(dgall[:].rearrange("p i m -> p (i m)"), dgalld[:])
            mcat = []
            for kc in range(NKC):
                m = maskp.tile([P, NM, QR], F8, tag=f"mc{kc}", name=f"mc{kc}")
                nc.sync.dma_start(
                    m[:].rearrange("p t q -> p (t q)"),
                    mcatd[kc * P:(kc + 1) * P, :],
                )
                mcat.append(m)
            cbt = win.tile([P, H * 8 + 2], F32, tag="cbt", name="cbt")
            nc.sync.dma_start(cbt[:], cbtd[:])
            selh = const.tile([4, 4 * 32], F32, tag="selh", name="selh")
            nc.sync.dma_start(selh[:], selhd[:])
            ident32 = const.tile([P, P], F32, tag="i32", name="i32")
            make_identity(nc, ident32[:])

            def dg_pair(h, j):   # lhsT [128, 2, 128] for thresholds 2j+1, 2j+2
                return dgall[:, h * NM + 2 * j: h * NM + 2 * j + 2, :]

            def dg_single(h):    # lhsT [128, 128] for threshold 15
                return dgall[:, h * NM + NM - 1, :]

            def mk_pair(kc, j):  # rhs [128, 2, 512]
                return mcat[kc][:, 2 * j:2 * j + 2, :]

            def mk_single(kc):   # rhs [128, 512]
                return mcat[kc][:, NM - 1, :]

            # ---------------- projections ----------------
            # scratch psum rotates over the 4 score banks (free until the
            # main loop) so head-split copies overlap the next matmul
            _scr = [0]

            def scratch_ps(cols):
                i = _scr[0] % 4
                _scr[0] += 1
                t = psc.tile([P, QR], F32, tag=f"sc{i}", name=f"sc{i}")
                return t[:, 0:cols]

            def hcopy(dst, src_ap, i):
                # alternate head-split copies between Vector and Scalar
                if i % 2 == 0:
                    nc.vector.tensor_copy(dst, src_ap)
                else:
                    nc.scalar.copy(dst, src_ap)

            KT_sb = [acts.tile([DH, N], F16, tag=f"kth{h}", name=f"kth{h}") for h in range(H)]
            QT_sb = [acts.tile([DH, QR], F16, tag=f"qth{h}", name=f"qth{h}") for h in range(H)]
            for hc in range(NDC):
                for nb in range(N // 512):
                    ps = scratch_ps(512)
                    for dc in range(NDC):
                        nc.tensor.matmul(
                            ps[:],
                            w_sb["k", dc][:, hc * P:(hc + 1) * P],
                            xT_sb[dc][:, nb * 512:(nb + 1) * 512],
                            start=(dc == 0), stop=(dc == NDC - 1),
                        )
                    for hr in range(4):
                        hcopy(KT_sb[4 * hc + hr][:, nb * 512:(nb + 1) * 512],
                              ps[32 * hr:32 * hr + 32, :], hr)
                ps = scratch_ps(QR)
                for dc in range(NDC):
                    nc.tensor.matmul(
                        ps[:],
                        w_sb["q", dc][:, hc * P:(hc + 1) * P],
                        xTq_sb[dc][:],
                        start=(dc == 0), stop=(dc == NDC - 1),
                    )
                for hr in range(4):
                    hcopy(QT_sb[4 * hc + hr][:], ps[32 * hr:32 * hr + 32, :], hr)

            # V_aug[k, 33h+d] fp16, col 33h+32 = ones
            V_sb = [acts.tile([P, 33 * H], F16, tag=f"v{kc}", name=f"v{kc}") for kc in range(NKC)]
            for kc in range(NKC):
                ps = scratch_ps(D)
                for dc in range(NDC):
                    nc.tensor.matmul(
                        ps[:],
                        xT_sb[dc][:, kc * P:(kc + 1) * P],
                        w_sb["v", dc][:],
                        start=(dc == 0), stop=(dc == NDC - 1),
                    )
                v3 = V_sb[kc][:].rearrange("p (h x) -> p h x", x=33)
                nc.scalar.copy(
                    v3[:, :, 0:32], ps[:].rearrange("p (h d) -> p h d", d=DH)
                )
                nc.vector.memset(v3[:, :, 32:33], 1.0)

            # NUM psum: 4 banks, 2 heads per bank at row offsets 0/64
            num_ps = [pnum.tile([P, QR], F32, tag=f"num{j}", name=f"num{j}") for j in range(4)]

            def num_slice(h, rows):
                j, i = divmod(h, 2)
                return num_ps[j][64 * i: 64 * i + rows, :]

            # ---------------- main loop: groups of key chunks ------------
            # denominators gathered as each head finishes (engines can't
            # write partition base 1..7 -> stage at partition 0, tiny DMA)
            zall = [misc.tile([4, QR], F32, tag=f"zall{i}", name=f"zall{i}")
                    for i in range(2)]
            zinv = {}
            for g, kcs in enumerate(([0, 1], [2, 3], [4, 5], [6, 7])):
                for h in range(H):
                    sc = {}
                    for gi, kc in enumerate(kcs):
                        tg = 2 * (g % 2) + gi
                        ps = psc.tile([P, QR], F32, tag=f"sc{tg}", name=f"sc{tg}")
                        nc.tensor.matmul(
                            ps[:],
                            KT_sb[h][:, kc * P:(kc + 1) * P],
                            QT_sb[h][:],
                            start=True, stop=False,
                        )
                        sc[kc] = ps
                    # kc-inner so the stationary fp8 diag is reused
                    for j in range(NP):
                        for kc in kcs:
                            nc.tensor.matmul(
                                sc[kc][:], dg_pair(h, j), mk_pair(kc, j),
                                start=False, stop=False,
                                perf_mode=mybir.MatmulPerfMode.DoubleRow,
                            )
                    for kc in kcs:
                        nc.tensor.matmul(
                            sc[kc][:], dg_single(h), mk_single(kc),
                            start=False, stop=True,
                        )
                    for kc in kcs:
                        e = epool.tile([P, QR], F16, tag="e", name="e")
                        nc.scalar.activation(
                            e[:], sc[kc][:], mybir.ActivationFunctionType.Exp,
                            bias=cbt[:, 8 * h + kc: 8 * h + kc + 1],
                            scale=float(SCALE),
                        )
                        nc.tensor.matmul(
                            num_slice(h, 33),
                            V_sb[kc][:, 33 * h: 33 * h + 33],
                            e[:],
                            start=(kc == 0), stop=(kc == NKC - 1),
                        )
                    if kcs[-1] == NKC - 1:
                        zr = misc.tile([1, QR], F32, tag=f"zr{h}", name=f"zr{h}")
                        nc.scalar.copy(zr[:], num_slice(h, 33)[32:33, :])
                        nc.sync.dma_start(zall[h // 4][h % 4:h % 4 + 1, :], zr[:])
                        if h % 4 == 3:
                            # this half's denominators are complete: its
                            # reciprocal overlaps the remaining heads
                            half = h // 4
                            zeps = misc.tile([4, QR], F32, tag=f"ze{half}", name=f"ze{half}")
                            nc.vector.tensor_scalar(
                                zeps[:], zall[half][:], 1e-30, None,
                                op0=mybir.AluOpType.add,
                            )
                            zi = misc.tile([4, QR], F32, tag=f"zi{half}", name=f"zi{half}")
                            nc.vector.reciprocal(zi[:], zeps[:])
                            zinv[half] = zi

            # ---------------- normalize + out-projection ----------------
            An = [outp.tile([P, QR], F16, tag=f"an{c}", name=f"an{c}") for c in range(NDC)]
            for h in range(H):
                hc, hr = divmod(h, 4)
                rsl = slice(32 * hr, 32 * hr + 32)
                rp = scratch_ps(QR)[0:32, :]
                nc.tensor.matmul(
                    rp[:], selh[:, 32 * (h % 4):32 * (h % 4) + 32],
                    zinv[h // 4][:],
                    start=True, stop=True,
                )
                rp_sb = misc.tile([32, QR], F32, tag="rp_sb", name="rp_sb")
                nc.vector.tensor_copy(rp_sb[:], rp[:])
                nc.vector.tensor_tensor(
                    An[hc][rsl, :], num_slice(h, 32), rp_sb[:],
                    op=mybir.AluOpType.mult,
                )

            oT = []
            for mc in range(NDC):
                ps = scratch_ps(QR)
                for cc in range(NDC):
                    nc.tensor.matmul(
                        ps[:],
                        w_sb["o", cc][:, mc * P:(mc + 1) * P],
                        An[cc][:],
                        start=(cc == 0), stop=(cc == NDC - 1),
                    )
                ot = outp.tile([P, QR], F32, tag=f"ot{mc}", name=f"ot{mc}")
                nc.scalar.add(ot[:], ps[:], cbt[:, 64 + mc:65 + mc])
                oT.append(ot)

            # transpose out^T [dm, q] -> out [q, dm] and DMA
            for qb in range(QR // P):
                osb = outp.tile([P, D], F32, tag=f"osb{qb % 2}", name=f"osb{qb % 2}")
                for mc in range(NDC):
                    tp = scratch_ps(P)
                    nc.tensor.transpose(
                        tp[:], oT[mc][:, qb * P:(qb + 1) * P], ident32[:]
                    )
                    hcopy(osb[:, mc * P:(mc + 1) * P], tp[:], mc)
                nc.sync.dma_start(out[qb * P:(qb + 1) * P, :], osb[:])

    if not nc.is_finalized():
        nc.finalize()
    return nc


def _prep_inputs(x, z_matrix, key_mask, Wq, bq, Wk, bk, Wv, bv, Wo, bo, z_emb,
                 **_unused):
    f32, f16 = np.float32, np.float16
    assert np.all(np.asarray(bq) == 0) and np.all(np.asarray(bk) == 0), (
        "nonzero bq/bk not supported by this kernel build"
    )
    z_emb = np.asarray(z_emb, dtype=f32)

    # weight tiles concatenated per partition: [P, (kqvo x c), D]
    wcat = np.empty((P, 8, D), dtype=f16)
    for i, W in enumerate((Wk, Wq, Wv, Wo)):
        WT = np.asarray(W, dtype=f32).T
        for c in range(2):
            wcat[:, 2 * i + c, :] = WT[c * P:(c + 1) * P, :].astype(f16)
    wcatd = np.ascontiguousarray(wcat.reshape(P, 8 * D))

    # fp8 staircase diagonals, all (h, t) tiles in one row-concat tensor
    q = _staircase_q(z_emb)
    dgall = np.zeros((P, H * NM, P), dtype=np.float32)
    ii = np.arange(P)
    for h in range(H):
        for t in range(NM):
            dgall[ii, h * NM + t, ii] = q[h, t]
    dgalld = np.ascontiguousarray(dgall.reshape(P, H * NM * P)).astype(F8NP)

    selhd = np.zeros((4, 4 * 32), dtype=f32)
    for h in range(4):
        selhd[h, 32 * h:32 * h + 32] = 1.0

    bo_eff = (np.asarray(Wo) @ np.asarray(bv) + np.asarray(bo)).astype(f32)

    in_maps = []
    for core in range(NCORES):
        b, half = divmod(core, 2)
        q0 = half * QR
        xb = np.asarray(x[b], dtype=f32)                    # [N, D]
        xT_ = np.ascontiguousarray(xb.T.astype(f16))        # [D, N]
        xTq_ = np.ascontiguousarray(xb[q0:q0 + QR, :].T.astype(f16))
        # threshold masks from bin indices, shipped as fp8 0/1
        zb_f = np.asarray(z_matrix[b], dtype=f32) * np.float32(NB / MAX_Z)
        zb_i = np.clip(zb_f.astype(np.int32), 0, NB - 1)
        idxT = zb_i.T[:, q0:q0 + QR]                        # [N, QR] int32
        one = np.uint8(np.float32(1.0).astype(F8NP).view(np.uint8))
        mcat_u8 = np.zeros((N, NM, QR), dtype=np.uint8)
        for t in range(NM):
            mcat_u8[:, t, :][idxT >= t + 1] = one
        mcatd = np.ascontiguousarray(mcat_u8.reshape(N, NM * QR)).view(F8NP)
        # exp-bias rows + folded output bias, one [P, 66] f32 tensor
        kma = np.asarray(key_mask[b]).astype(f32) * np.float32(-1e30)  # [N]
        cbt = np.empty((P, H * 8 + 2), dtype=f32)
        for h in range(H):
            for kc in range(8):
                cbt[:, 8 * h + kc] = kma[kc * P:(kc + 1) * P] + z_emb[0, h]
        cbt[:, 64] = bo_eff[0:P]
        cbt[:, 65] = bo_eff[P:2 * P]
        in_maps.append({
            "xT": xT_, "xTq": xTq_, "mcatd": mcatd,
            "wcatd": wcatd, "dgalld": dgalld,
            "cbtd": np.ascontiguousarray(cbt), "selhd": selhd,
        })
    return in_maps


def kernel(**inputs) -> np.ndarray:
    z_emb = np.asarray(inputs["z_emb"], dtype=np.float32)
    key = z_emb.tobytes()
    if key not in _CACHE:
        _CACHE[key] = _build(z_emb)
    nc = _CACHE[key]

    in_maps = _prep_inputs(**inputs)
    res = run_bass_kernel_spmd(nc, in_maps, core_ids=list(range(NCORES)))
    full = np.empty((B, N, D), dtype=np.float32)
    for c in range(NCORES):
        b, half = divmod(c, 2)
        full[b, half * QR:(half + 1) * QR, :] = res.results[c]["out"]
    return full
